# revision 9
# baseline (speedup 1.0000x reference)
"""Trainium2 Bass kernel for nn_DigitConvolutionalModel.

Model: x(B,784) -> reshape 28x28 -> 3x3 valid cross-correlation (kernel is an
input) -> flatten 676 -> Linear(676,128)+ReLU -> Linear(128,10).

Strategy:
  * Fold the 3x3 conv into the first linear layer on the host: the conv is a
    linear map, so h = relu(x @ W1eff.T + b1) with W1eff (128, 784) built by
    scattering conv_w-weighted copies of w1 onto the 28x28 grid. The device
    kernel is then a plain 2-layer MLP over 784 features.
  * Pure data parallelism: batch 65536 split as 8192 rows per NeuronCore,
    weights replicated.
  * The per-core DMA wall with all 8 cores streaming is ~270 GB/s of
    SBUF-write bytes (per-SDMA-engine descriptor throughput), so x ships
    AND stages as int8: q = round(x/s), one global scale s = absmax/127
    folded exactly into the fp16 layer-1 weights. DMA writes 6.4 MB int8
    instead of 12.8 MB fp16. Measured end-to-end error 1.4e-2 of scale
    (gate 2e-2); inputs are deterministic (fixed seed).
  * The PE has no int8 mode, so each 1024-column unit is cast int8->fp16
    by a compute engine (DVE / GpSimd alternating) through the engine SBUF
    ports - a separate bandwidth domain from the DMA fabric. The relu+b1
    epilogue runs on the scalar engine (activation Relu with bias), the
    b2-add evacuation on DVE, so no single engine is saturated.
  * x is packed per DMA block with each partition's block data one
    contiguous HBM run (per-descriptor overhead throttles small
    descriptors). Weights load first on the same sync ring as x - per-ring
    FIFO guarantees they land before block 0 (a separate queue would
    round-robin against the bulk x stream and arrive ~20 us late).
    Biases ride in the weight tile as fp16, widened to fp32 by one DVE op.
  * Layer-2 work for unit u is emitted between unit u+1's layer-1 matmuls
    so the PE FIFO never waits on the epilogue.
"""

from contextlib import ExitStack

import numpy as np

B = 65536
H = W = 28
K = 3
CH = CW = 26
FEAT = H * W          # 784
HID = 128
OUT = 10
NCORES = 8
BC = B // NCORES      # 8192 rows per core

KC = 112              # contraction-chunk partition size
KCH = 7               # chunks: 7 * 112 = 784
NT = 512              # batch rows per matmul (one PSUM bank fp32)
UC = 1024             # batch rows per compute unit (2 PSUM banks)
WCOL = KCH * HID      # 896 w1t columns in the packed weight tile
# wpk columns: [w1t 0:896][w2t 896:906][b1 906][b2 907]
WTOT = WCOL + OUT + 2

VARIANT = "i8"

_NC_CACHE = {}


def _blocks(bc):
    if bc == 8192:
        blocks = [2048, 2048, 2048, 1024, 512, 512]
    else:
        blocks = [min(1024, bc - o) for o in range(0, bc, 1024)]
    assert sum(blocks) == bc
    return blocks


def _build_nc(bc, variant):
    from concourse import bacc
    import concourse.mybir as mybir
    import concourse.tile as tile

    f32 = mybir.dt.float32
    f16 = mybir.dt.float16
    i8 = mybir.dt.int8
    blocks = _blocks(bc)

    nc = bacc.Bacc(
        "TRN2",
        target_bir_lowering=False,
        debug=False,
        enable_asserts=False,
        num_devices=NCORES,
    )
    # [112, 7*bc] with per-block column groups: block b at columns
    # [7*off_b, 7*(off_b+xb)), chunk-major inside the block so each
    # partition's block data is one contiguous HBM run
    xT = nc.dram_tensor("xT", [KC, KCH * bc], i8, kind="ExternalInput").ap()
    wpk = nc.dram_tensor("wpk", [HID, WTOT], f16, kind="ExternalInput").ap()
    outT = nc.dram_tensor("outT", [OUT, bc], f32, kind="ExternalOutput").ap()

    with ExitStack() as ctx:
        tc = ctx.enter_context(tile.TileContext(nc))
        wpool = ctx.enter_context(tc.tile_pool(name="w", bufs=1))
        xqpool = ctx.enter_context(tc.tile_pool(name="xq", bufs=len(blocks)))
        xfpool = ctx.enter_context(tc.tile_pool(name="xf", bufs=4))
        hpool = ctx.enter_context(tc.tile_pool(name="h", bufs=3))
        opool = ctx.enter_context(tc.tile_pool(name="o", bufs=3))
        p1pool = ctx.enter_context(tc.tile_pool(name="p1", bufs=2, space="PSUM"))
        p2pool = ctx.enter_context(tc.tile_pool(name="p2", bufs=2, space="PSUM"))

        # weights first on the sync ring: FIFO within the ring means their
        # descriptors drain before block 0's
        ws = wpool.tile([HID, WTOT], f16)
        nc.sync.dma_start(ws[:], wpk[:])
        w2s = ws[:, WCOL : WCOL + OUT]
        bs = wpool.tile([HID, 2], f32)
        nc.vector.tensor_copy(bs[:], ws[:, WCOL + OUT : WCOL + OUT + 2])
        b1s = bs[:, 0:1]
        b2s = bs[0:OUT, 1:2]

        xq_list = []
        off = 0
        for blk, xb in enumerate(blocks):
            # [112, 7, xb]: per-partition layout is the same contiguous
            # 7*xb run the HBM slice has -> one descriptor per partition
            xq = xqpool.tile([KC, KCH, xb], i8, tag="xq", name=f"xq_{blk}")
            nc.sync.dma_start(xq[:], xT[:, KCH * off : KCH * (off + xb)])
            xq_list.append(xq)
            off += xb

        add = mybir.AluOpType.add
        relu = mybir.ActivationFunctionType.Relu

        # units: (blk, xb, unit column offset in block, unit cols, global off)
        units = []
        off = 0
        for blk, xb in enumerate(blocks):
            for u0 in range(0, xb, UC):
                units.append((blk, xb, u0, min(UC, xb - u0), off + u0))
            off += xb

        def emit_cast(uidx):
            """int8 -> fp16 for one unit on a compute engine. The source is
            7 strided runs (chunk stride xb inside the block); the dest is
            the unit-local chunk-major fp16 tile."""
            blk, xb, u0, uc, _ = units[uidx]
            xq = xq_list[blk]
            xf = xfpool.tile([KC, KCH, uc], f16, tag="xf", name=f"xf_{uidx}")
            eng = nc.vector if uidx % 2 == 0 else nc.gpsimd
            eng.tensor_copy(xf[:], xq[:, :, u0 : u0 + uc])
            return xf

        def emit_l1(uidx, xf):
            blk, xb, u0, uc, _ = units[uidx]
            p1 = p1pool.tile([HID, uc], f32, tag="p1", name=f"p1_{uidx}")
            for t0 in range(0, uc, NT):
                nt = min(NT, uc - t0)
                for c in range(KCH):
                    nc.tensor.matmul(
                        p1[:, t0 : t0 + nt],
                        ws[0:KC, c * HID : (c + 1) * HID],
                        xf[:, c, t0 : t0 + nt],
                        start=(c == 0),
                        stop=(c == KCH - 1),
                    )
            return p1

        def emit_l2(uidx, p1):
            blk, xb, u0, uc, goff = units[uidx]
            hs = hpool.tile([HID, uc], f16, tag="hs", name=f"hs_{uidx}")
            # relu + b1 on the scalar engine: out = relu(p1 + b1)
            nc.scalar.activation(hs[:], p1[:], relu, bias=b1s)
            p2 = p2pool.tile([OUT, uc], f32, tag="p2", name=f"p2_{uidx}")
            for t0 in range(0, uc, NT):
                nt = min(NT, uc - t0)
                nc.tensor.matmul(
                    p2[:, t0 : t0 + nt], w2s, hs[:, t0 : t0 + nt],
                    start=True, stop=True,
                )
            os_ = opool.tile([OUT, uc], f32, tag="os", name=f"os_{uidx}")
            nc.vector.tensor_scalar_add(os_[:], p2[:], b2s)
            nc.sync.dma_start(outT[:, goff : goff + uc], os_[:])

        prev = None
        for uidx in range(len(units)):
            xf = emit_cast(uidx)
            p1 = emit_l1(uidx, xf)
            if prev is not None:
                emit_l2(*prev)
            prev = (uidx, p1)
        emit_l2(*prev)

    nc.compile()
    return nc


def get_nc(bc=BC, variant=VARIANT):
    key = (bc, variant)
    if key not in _NC_CACHE:
        _NC_CACHE[key] = _build_nc(bc, variant)
    return _NC_CACHE[key]


def _pack_xT(shard, blocks):
    """[bc, 784] row-major shard -> [112, 7*bc] per-block-contiguous."""
    parts = []
    off = 0
    for xb in blocks:
        sub = shard[off : off + xb]  # [xb, 784]
        # [xb, 7, 112] -> [112, 7, xb] -> [112, 7*xb]
        parts.append(sub.reshape(xb, KCH, KC).transpose(2, 1, 0).reshape(KC, KCH * xb))
        off += xb
    return np.ascontiguousarray(np.concatenate(parts, axis=1))


def _host_prep(x, conv_w, w1, b1, w2, b2, variant):
    """Fold conv into layer-1 weights, quantize x, lay out device inputs."""
    x = np.asarray(x, dtype=np.float32)
    conv_w = np.asarray(conv_w, dtype=np.float32)
    w1 = np.asarray(w1, dtype=np.float32)
    b1 = np.asarray(b1, dtype=np.float32)
    w2 = np.asarray(w2, dtype=np.float32)
    b2 = np.asarray(b2, dtype=np.float32)

    w1_img = w1.reshape(HID, CH, CW)
    w1eff = np.zeros((HID, H, W), dtype=np.float32)
    for di in range(K):
        for dj in range(K):
            w1eff[:, di : di + CH, dj : dj + CW] += conv_w[di, dj] * w1_img
    w1eff = w1eff.reshape(HID, FEAT)

    s = float(np.abs(x).max()) / 127.0
    xq = np.clip(np.round(x * (1.0 / s)), -127, 127).astype(np.int8)
    w1dev = (w1eff * s).astype(np.float16)

    # w1t layout [112, 7*128]: chunk c partition p holds feature c*112+p
    w1t_host = (
        w1dev.astype(np.float32)
        .T.reshape(KCH, KC, HID)
        .transpose(1, 0, 2)
        .reshape(KC, KCH * HID)
    )
    wpk_host = np.zeros((HID, WTOT), dtype=np.float32)
    wpk_host[0:KC, 0:WCOL] = w1t_host
    wpk_host[:, WCOL : WCOL + OUT] = w2.T
    wpk_host[:, WCOL + OUT] = b1
    wpk_host[0:OUT, WCOL + OUT + 1] = b2
    wpk_host = np.ascontiguousarray(wpk_host).astype(np.float16)

    blocks = _blocks(BC)
    in_maps = []
    for c in range(NCORES):
        in_maps.append(
            {
                "xT": _pack_xT(xq[c * BC : (c + 1) * BC], blocks),
                "wpk": wpk_host,
            }
        )
    return in_maps


def run(x, conv_w, w1, b1, w2, b2, trace=False, variant=VARIANT):
    from concourse.bass_utils import run_bass_kernel_spmd

    in_maps = _host_prep(x, conv_w, w1, b1, w2, b2, variant)
    nc = get_nc(BC, variant)
    res = run_bass_kernel_spmd(nc, in_maps, list(range(NCORES)), trace=trace)
    outT = np.concatenate([r["outT"] for r in res.results], axis=1)  # [10, B]
    return np.ascontiguousarray(outT.T), res


def kernel(x, conv_w, w1, b1, w2, b2):
    out, _ = run(x, conv_w, w1, b1, w2, b2)
    return out


# revision 10
# speedup vs baseline: 1.8103x; 1.8103x over previous
"""Trainium2 Bass kernel for nn_DigitConvolutionalModel.

Model: x(B,784) -> reshape 28x28 -> 3x3 valid cross-correlation (kernel is an
input) -> flatten 676 -> Linear(676,128)+ReLU -> Linear(128,10).

Strategy:
  * Fold the 3x3 conv into the first linear layer on the host: the conv is a
    linear map, so h = relu(x @ W1eff.T + b1) with W1eff (128, 784) built by
    scattering conv_w-weighted copies of w1 onto the 28x28 grid. The device
    kernel is then a plain 2-layer MLP over 784 features.
  * Pure data parallelism: batch 65536 split as 8192 rows per NeuronCore,
    weights replicated.
  * The binding resource is per-SDMA-engine byte throughput (~17.5 GB/s x
    16 engines ~ 280 GB/s per core with all 8 cores streaming, regardless
    of dtype or descriptor size >= 14 KB). The PE only eats fp16/bf16/fp8
    moving operands and fp8 quantization of x fails the accuracy gate
    (3.1e-2 vs 2e-2), while engine-side int8->fp16 casts run at 1x DVE
    mode (~60 us of engine work) - so fp16 x (12.85 MB/core, ~45 us
    stream) is the optimum; measured error 5e-4 of scale.
  * x is shipped packed per 1024-row DMA block with each partition's block
    data one contiguous HBM run: 112 descriptors of 14 KB per block (the
    per-descriptor sweet spot; 4 KB descriptors throttle at ~240 GB/s).
  * Weights load first on the SAME sync ring as x - per-engine FIFO within
    a ring guarantees they land before block 0 (a separate queue would
    round-robin against the bulk x stream and arrive ~20 us late, stalling
    the first matmul and, via DMA-lane sem reuse, later x descriptor
    generation). Biases ride in the weight tile as fp16 and are widened to
    fp32 by one DVE op. Output stores ride the scalar ring so a waiting
    store never head-of-line-blocks an x load.
  * relu+b1 runs on the scalar engine (ACTIVATE Relu with bias AP,
    ~1 us per 1024-block), the b2-add evacuation on DVE; layer-2 work for
    block b is emitted between block b+1's layer-1 matmuls so the PE FIFO
    never waits on the epilogue. Compute tracks the DMA closely enough
    that the PE never idles past the ~3.4 us HAM window.
"""

from contextlib import ExitStack

import numpy as np

B = 65536
H = W = 28
K = 3
CH = CW = 26
FEAT = H * W          # 784
HID = 128
OUT = 10
NCORES = 8
BC = B // NCORES      # 8192 rows per core

KC = 112              # contraction-chunk partition size
KCH = 7               # chunks: 7 * 112 = 784
NT = 512              # batch rows per matmul (one PSUM bank fp32)
WCOL = KCH * HID      # 896 w1t columns in the packed weight tile
# wpk columns: [w1t 0:896][w2t 896:906][b1 906][b2 907]
WTOT = WCOL + OUT + 2

VARIANT = "f16"

_NC_CACHE = {}


def _blocks(bc):
    # 1024-row blocks (14 KB descriptors); small final blocks so the
    # post-stream compute tail is short
    if bc == 8192:
        blocks = [1024] * 7 + [512, 512]
    else:
        blocks = [min(1024, bc - o) for o in range(0, bc, 1024)]
    assert sum(blocks) == bc
    return blocks


def _build_nc(bc, variant):
    from concourse import bacc
    import concourse.mybir as mybir
    import concourse.tile as tile

    f32 = mybir.dt.float32
    f16 = mybir.dt.float16
    blocks = _blocks(bc)

    nc = bacc.Bacc(
        "TRN2",
        target_bir_lowering=False,
        debug=False,
        enable_asserts=False,
        num_devices=NCORES,
    )
    # [112, 7*bc] with per-block column groups: block b at columns
    # [7*off_b, 7*(off_b+xb)), chunk-major inside the block so each
    # partition's block data is one contiguous HBM run
    xT = nc.dram_tensor("xT", [KC, KCH * bc], f16, kind="ExternalInput").ap()
    wpk = nc.dram_tensor("wpk", [HID, WTOT], f16, kind="ExternalInput").ap()
    outT = nc.dram_tensor("outT", [OUT, bc], f32, kind="ExternalOutput").ap()

    with ExitStack() as ctx:
        tc = ctx.enter_context(tile.TileContext(nc))
        wpool = ctx.enter_context(tc.tile_pool(name="w", bufs=1))
        xpool = ctx.enter_context(tc.tile_pool(name="x", bufs=len(blocks)))
        hpool = ctx.enter_context(tc.tile_pool(name="h", bufs=3))
        opool = ctx.enter_context(tc.tile_pool(name="o", bufs=3))
        p1pool = ctx.enter_context(tc.tile_pool(name="p1", bufs=2, space="PSUM"))
        p2pool = ctx.enter_context(tc.tile_pool(name="p2", bufs=2, space="PSUM"))

        # weights first on the sync ring: FIFO within the ring means their
        # descriptors drain before block 0's
        ws = wpool.tile([HID, WTOT], f16)
        nc.sync.dma_start(ws[:], wpk[:])
        w2s = ws[:, WCOL : WCOL + OUT]
        bs = wpool.tile([HID, 2], f32)
        nc.vector.tensor_copy(bs[:], ws[:, WCOL + OUT : WCOL + OUT + 2])
        b1s = bs[:, 0:1]
        b2s = bs[0:OUT, 1:2]

        xs_list = []
        off = 0
        for blk, xb in enumerate(blocks):
            # [112, 7, xb]: per-partition layout is the same contiguous
            # 7*xb run the HBM slice has -> one descriptor per partition
            xs = xpool.tile([KC, KCH, xb], f16, tag="xs", name=f"xs_{blk}")
            nc.sync.dma_start(xs[:], xT[:, KCH * off : KCH * (off + xb)])
            xs_list.append(xs)
            off += xb

        add = mybir.AluOpType.add
        relu = mybir.ActivationFunctionType.Relu

        offs = []
        off = 0
        for xb in blocks:
            offs.append(off)
            off += xb

        def emit_l1(blk):
            xb = blocks[blk]
            xs = xs_list[blk]
            p1 = p1pool.tile([HID, xb], f32, tag="p1", name=f"p1_{blk}")
            for t0 in range(0, xb, NT):
                nt = min(NT, xb - t0)
                for c in range(KCH):
                    nc.tensor.matmul(
                        p1[:, t0 : t0 + nt],
                        ws[0:KC, c * HID : (c + 1) * HID],
                        xs[:, c, t0 : t0 + nt],
                        start=(c == 0),
                        stop=(c == KCH - 1),
                    )
            return p1

        def emit_l2(blk, p1):
            xb = blocks[blk]
            hs = hpool.tile([HID, xb], f16, tag="hs", name=f"hs_{blk}")
            # relu + b1 on the scalar engine: out = relu(p1 + b1)
            nc.scalar.activation(hs[:], p1[:], relu, bias=b1s)
            p2 = p2pool.tile([OUT, xb], f32, tag="p2", name=f"p2_{blk}")
            for t0 in range(0, xb, NT):
                nt = min(NT, xb - t0)
                nc.tensor.matmul(
                    p2[:, t0 : t0 + nt], w2s, hs[:, t0 : t0 + nt],
                    start=True, stop=True,
                )
            os_ = opool.tile([OUT, xb], f32, tag="os", name=f"os_{blk}")
            nc.vector.tensor_scalar_add(os_[:], p2[:], b2s)
            nc.scalar.dma_start(outT[:, offs[blk] : offs[blk] + xb], os_[:])

        prev = None
        for blk in range(len(blocks)):
            p1 = emit_l1(blk)
            if prev is not None:
                emit_l2(*prev)
            prev = (blk, p1)
        emit_l2(*prev)

    nc.compile()
    return nc


def get_nc(bc=BC, variant=VARIANT):
    key = (bc, variant)
    if key not in _NC_CACHE:
        _NC_CACHE[key] = _build_nc(bc, variant)
    return _NC_CACHE[key]


def _pack_xT(shard, blocks):
    """[bc, 784] row-major fp16 shard -> [112, 7*bc] per-block-contiguous."""
    parts = []
    off = 0
    for xb in blocks:
        sub = shard[off : off + xb]  # [xb, 784]
        # [xb, 7, 112] -> [112, 7, xb] -> [112, 7*xb]
        parts.append(sub.reshape(xb, KCH, KC).transpose(2, 1, 0).reshape(KC, KCH * xb))
        off += xb
    return np.ascontiguousarray(np.concatenate(parts, axis=1))


def _host_prep(x, conv_w, w1, b1, w2, b2, variant):
    """Fold conv into layer-1 weights and lay out per-core device inputs."""
    x = np.asarray(x, dtype=np.float32)
    conv_w = np.asarray(conv_w, dtype=np.float32)
    w1 = np.asarray(w1, dtype=np.float32)
    b1 = np.asarray(b1, dtype=np.float32)
    w2 = np.asarray(w2, dtype=np.float32)
    b2 = np.asarray(b2, dtype=np.float32)

    w1_img = w1.reshape(HID, CH, CW)
    w1eff = np.zeros((HID, H, W), dtype=np.float32)
    for di in range(K):
        for dj in range(K):
            w1eff[:, di : di + CH, dj : dj + CW] += conv_w[di, dj] * w1_img
    w1eff = w1eff.reshape(HID, FEAT)

    # w1t layout [112, 7*128]: chunk c partition p holds feature c*112+p
    w1t_host = (
        w1eff.T.reshape(KCH, KC, HID).transpose(1, 0, 2).reshape(KC, KCH * HID)
    )
    wpk_host = np.zeros((HID, WTOT), dtype=np.float32)
    wpk_host[0:KC, 0:WCOL] = w1t_host
    wpk_host[:, WCOL : WCOL + OUT] = w2.T
    wpk_host[:, WCOL + OUT] = b1
    wpk_host[0:OUT, WCOL + OUT + 1] = b2
    wpk_host = np.ascontiguousarray(wpk_host).astype(np.float16)

    blocks = _blocks(BC)
    xq = x.astype(np.float16)
    in_maps = []
    for c in range(NCORES):
        in_maps.append(
            {
                "xT": _pack_xT(xq[c * BC : (c + 1) * BC], blocks),
                "wpk": wpk_host,
            }
        )
    return in_maps


def run(x, conv_w, w1, b1, w2, b2, trace=False, variant=VARIANT):
    from concourse.bass_utils import run_bass_kernel_spmd

    in_maps = _host_prep(x, conv_w, w1, b1, w2, b2, variant)
    nc = get_nc(BC, variant)
    res = run_bass_kernel_spmd(nc, in_maps, list(range(NCORES)), trace=trace)
    outT = np.concatenate([r["outT"] for r in res.results], axis=1)  # [10, B]
    return np.ascontiguousarray(outT.T), res


def kernel(x, conv_w, w1, b1, w2, b2):
    out, _ = run(x, conv_w, w1, b1, w2, b2)
    return out
